# revision 28
# baseline (speedup 1.0000x reference)
"""Trainium2 Bass kernel for nn_LocalSwarmAggregator (sliding-window causal MHA).

Reference computation (fp32):
    q,k,v = x@Wq+bq, x@Wk+bk, x@Wv+bv          # [B,N,D] -> per-head [B,H,N,64]
    logits = q k^T / 8 + band_mask              # causal + 256-window
    out = softmax(logits) v                     # [B,H,N,64]
    y = concat_heads(out) @ Wo + bo             # [B,N,D]

Sharding over 8 cores: core c handles batch c//4 and heads 4*(c%4)..4*(c%4)+3
(tensor-parallel on the head dim of Wq/Wk/Wv and the row dim of Wo).  Each
core computes a partial y (bf16) for its batch; the host sums the 4 partials
per batch in fp32 and adds bo.  No cross-device communication.

v3: single software-pipelined loop over 512-query groups, all bf16 storage
and matmuls (fp32 PSUM).  The per-group steady state interleaves, on the PE:
QKV chains for group g (x^T is host-pre-transposed, so no x transposes),
S^T = K q^T for key tiles 4g-2..4g+1, AV for group g-1, and the output
projection for group g-2.  This keeps the scalar engine's exp stream (the
2nd-busiest engine) fully hidden under PE work, and keeps the PE dense so it
stays at its max p-state.  Host-side arrays are pre-arranged so every DMA is
contiguous per partition line (descriptor issue time, not bandwidth, was the
startup bottleneck).

PSUM (8 banks): acc ring 2 (QKV chains + out-proj blocks share one tag),
S^T 2x[128,2,512] = 4, AV [65,512] = 1, v-transpose [128,512]bf16 = 1.

Attention details: for key tile kt only queries 128*kt..128*kt+383 attend;
the valid band is r <= c <= r+256 for every kt -> one constant 0/1 bf16 mask
applied to P^T = exp(S^T/8) (logits are O(6): no row-max needed).  v^T is
transposed to v-natural on the PE and augmented with a ones column so the
softmax denominators fall out of the AV matmul (psum row 64).  The AV psum
is copied raw to SBUF immediately (numerator via DVE, denominator row via
the scalar engine to partition 0 - reciprocal_approx_fast misreads inputs at
a nonzero partition offset), freeing the single AV bank; reciprocal + gpsimd
partition_broadcast + multiply produce normalized U2 off the critical path.
"""

import os
from contextlib import ExitStack

import numpy as np

import concourse.bass as bass
import concourse.mybir as mybir
import concourse.tile as tile
from concourse import bacc
from concourse.bass_utils import run_bass_kernel_spmd
from concourse.masks import make_identity

F32 = mybir.dt.float32
N = 2048
D = 1024
HD = 64
WIN = 256
NPAIR = 2  # head pairs per core (4 heads)
NCH = D // 128  # 8 contraction chunks
NKT = N // 128  # 16 key tiles
SPAN = 384  # max query span per S^T key tile
QG = 512  # query group size
NQG = N // QG  # 4
SCALE = 1.0 / np.sqrt(HD)

_MM_DT_NAME = os.environ.get("BASS_MM_DT", "bfloat16")
MM_DT = getattr(mybir.dt, _MM_DT_NAME)

Exp = mybir.ActivationFunctionType.Exp
IS_GE = mybir.AluOpType.is_ge


def _span(kt):
    return min(SPAN, N - 128 * kt)


def _av_slices(kt):
    """For key tile kt return [(g, lo, hi, plo)]: query-group g consumes
    P^T[kt][:, lo:hi] into psum columns plo:plo+(hi-lo)."""
    span = _span(kt)
    out = []
    for g in range(NQG):
        lo = max(0, QG * g - 128 * kt)
        hi = min(span, QG * g + QG - 128 * kt)
        if lo < hi:
            out.append((g, lo, hi, 128 * kt + lo - QG * g))
    return out


def _group_kts(g):
    return [kt for kt in range(NKT) if any(s[0] == g for s in _av_slices(kt))]


def _s_kts(g):
    """Key tiles whose S^T is emitted in group g's stream (they need q/k
    columns up to 512*g+512, i.e. group <= g)."""
    if g < NQG:
        return [kt for kt in (4 * g - 2, 4 * g - 1, 4 * g, 4 * g + 1) if kt >= 0]
    return [14, 15]


def _emit(ctx: ExitStack, tc: tile.TileContext, aps, mm_dt, debug_taps=None):
    nc = tc.nc
    xTd, wq, wk, wv, wo, bq, bk, bv, out = aps
    MDT = mm_dt

    consts = ctx.enter_context(tc.tile_pool(name="consts", bufs=1))
    persist = ctx.enter_context(tc.tile_pool(name="persist", bufs=1))

    ident_f = consts.tile([128, 128], F32, tag="ident_f")
    make_identity(nc, ident_f)
    ident = consts.tile([128, 128], MDT, tag="ident")
    nc.vector.tensor_copy(ident, ident_f)

    mask_f = consts.tile([128, SPAN], F32, tag="mask_f")
    nc.gpsimd.memset(mask_f, 1.0)
    nc.gpsimd.affine_select(
        out=mask_f, in_=mask_f, compare_op=IS_GE, fill=0.0,
        base=0, pattern=[[1, SPAN]], channel_multiplier=-1,
    )  # keep c - r >= 0
    nc.gpsimd.affine_select(
        out=mask_f, in_=mask_f, compare_op=IS_GE, fill=0.0,
        base=WIN, pattern=[[-1, SPAN]], channel_multiplier=1,
    )  # keep r - c + WIN >= 0
    mask2 = consts.tile([128, 2, SPAN], MDT, tag="mask2")
    nc.vector.tensor_copy(mask2[:, 0, :], mask_f)
    nc.vector.tensor_copy(mask2[:, 1, :], mask_f)

    onesf = consts.tile([128, 1], F32, tag="onesf")
    nc.vector.memset(onesf, 1.0)
    onesb = consts.tile([1, HD], MDT, tag="onesb")
    nc.vector.memset(onesb, 1.0)
    zf = consts.tile([1, QG], F32, tag="zf")
    nc.vector.memset(zf, 0.0)
    zcol = consts.tile([1, 65], MDT, tag="zcol")
    nc.vector.tensor_copy(zcol, zf[:, 0:65])
    zrow = consts.tile([1, QG], MDT, tag="zrow")
    nc.vector.tensor_copy(zrow, zf)

    qT = persist.tile([128, NPAIR, N], MDT, tag="qT")
    kT = persist.tile([128, NPAIR, N], MDT, tag="kT")
    vaug = [persist.tile([128, NKT, 2, HD + 1], MDT, tag=f"vaug{p}",
                         name=f"vaug{p}")
            for p in range(NPAIR)]
    U2 = persist.tile([128, NPAIR, N], MDT, tag="U2")

    # ---- input DMAs: xT group 0 first on the sync queue; weights/biases on
    # the scalar engine's queue so their descriptor issue doesn't delay xT.
    # One xT tile per query group: dependency tracking is tile-granular, so a
    # shared tile would serialize later xT DMAs behind earlier groups' reads.
    xT_sb = [consts.tile([128, NCH, QG], MDT, tag=f"xT{g}", name=f"xT{g}")
             for g in range(NQG)]
    nc.sync.dma_start(out=xT_sb[0], in_=xTd[:, 0, :, :])
    w_sb = {}
    b_sb = {}
    for nm, wap in (("q", wq), ("k", wk), ("v", wv)):
        t = consts.tile([128, NCH, 2 * 128], MDT, tag=f"w{nm}", name=f"w{nm}")
        nc.scalar.dma_start(out=t, in_=wap)
        w_sb[nm] = t
    for nm, bap in (("q", bq), ("k", bk), ("v", bv)):
        t = consts.tile([128, NPAIR], F32, tag=f"b{nm}", name=f"b{nm}")
        nc.scalar.dma_start(out=t, in_=bap)
        b_sb[nm] = t
    wo_sb = consts.tile([128, NPAIR, D], MDT, tag="wo")
    nc.scalar.dma_start(out=wo_sb, in_=wo)

    acc = ctx.enter_context(tc.tile_pool(name="acc", bufs=2, space="PSUM"))
    psS = ctx.enter_context(tc.tile_pool(name="psS", bufs=2, space="PSUM"))
    psAV = ctx.enter_context(tc.tile_pool(name="psAV", bufs=1, space="PSUM"))
    psT = ctx.enter_context(tc.tile_pool(name="psT", bufs=1, space="PSUM"))
    vt_pool = ctx.enter_context(tc.tile_pool(name="vt", bufs=2))
    pt_pool = ctx.enter_context(tc.tile_pool(name="pt", bufs=16))
    u2r_pool = ctx.enter_context(tc.tile_pool(name="u2r", bufs=3))
    rb_pool = ctx.enter_context(tc.tile_pool(name="rb", bufs=3))
    ob_pool = ctx.enter_context(tc.tile_pool(name="ob", bufs=4))

    for pair in range(NPAIR):
        nc.vector.tensor_copy(
            vaug[pair][:, :, :, HD:HD + 1],
            onesf.broadcast_to((128, NKT, 2, 1)),
        )

    pts = {}
    vts = {}
    nblk = [0]  # out-proj block counter (for cast-engine alternation)

    def qkv_chain(g, pair, nm):
        gsl = slice(QG * g, QG * (g + 1))
        psq = acc.tile([128, QG], F32, tag="acc")
        for c in range(NCH):
            nc.tensor.matmul(
                psq,
                w_sb[nm][:, c, 128 * pair:128 * (pair + 1)],
                xT_sb[g][:, c, :],
                start=(c == 0), stop=(c == NCH - 1),
            )
        if nm == "v":
            vt = vts[g]
            dst = vt[:, pair, :]
        else:
            dst = (qT if nm == "q" else kT)[:, pair, gsl]
        nc.vector.tensor_scalar_add(dst, psq, b_sb[nm][:, pair:pair + 1])

    def vtrans(g, pair):
        vt = vts[g]
        pst = psT.tile([128, QG], MDT, tag="pst")
        for j in range(4):
            nc.tensor.transpose(
                pst[:, 128 * j:128 * (j + 1)],
                vt[:, pair, 128 * j:128 * (j + 1)], ident,
            )
        nc.vector.tensor_copy(
            vaug[pair][:, 4 * g:4 * (g + 1), :, 0:HD],
            pst.rearrange("p (j h d) -> p j h d", j=4, h=2),
        )

    def s_unit(kt, pair):
        span = _span(kt)
        q0 = 128 * kt
        pss = psS.tile([128, 2, QG], F32, tag="pss")
        for h in range(2):
            hb = 64 * h
            nc.tensor.matmul(
                pss[:, h, 0:span],
                kT[hb:hb + 64, pair, q0:q0 + 128],
                qT[hb:hb + 64, pair, q0:q0 + span],
                start=True, stop=True,
            )
        pt = pt_pool.tile([128, 2, SPAN], MDT, tag="pt")
        nc.scalar.activation(
            pt[:, :, 0:span], pss[:, :, 0:span], Exp, scale=SCALE)
        nc.vector.tensor_mul(
            pt[:, :, 0:128], pt[:, :, 0:128], mask2[:, :, 0:128])
        if span > WIN:
            nc.vector.tensor_mul(
                pt[:, :, WIN:span], pt[:, :, WIN:span],
                mask2[:, :, WIN:span])
        if debug_taps is not None and pair == 0 and kt < 4:
            nc.sync.dma_start(out=debug_taps[0][:, kt, :, :], in_=pt)
        pts[(pair, kt)] = pt

    def av_unit(g, pair, h, pe_bcast=False):
        psav = psAV.tile([65, QG], F32, tag="psav")
        nc.tensor.matmul(psav, zcol, zrow, start=True, stop=False)
        kts = _group_kts(g)
        for i, kt in enumerate(kts):
            (lo, hi, plo) = next(
                (s[1], s[2], s[3]) for s in _av_slices(kt) if s[0] == g)
            nc.tensor.matmul(
                psav[:, plo:plo + (hi - lo)],
                vaug[pair][:, kt, h, :],
                pts[(pair, kt)][:, h, lo:hi],
                start=False, stop=(i == len(kts) - 1),
            )
        u2rn = u2r_pool.tile([64, QG], F32, tag="u2rn")
        nc.vector.tensor_copy(u2rn, psav[0:64, :])
        rt0 = rb_pool.tile([1, QG], F32, tag="rt0")
        nc.scalar.copy(rt0, psav[64:65, :])
        if pe_bcast:
            # drain-only: broadcast 1/denom via a rank-1 PE matmul into a
            # (then-idle) S-pool bank — skips the slow gpsimd hop.
            rtmpf = rb_pool.tile([1, QG], F32, tag="rtmp")
            nc.vector.reciprocal_approx_fast(out=rtmpf, in_=rt0)
            rtmpb = rb_pool.tile([1, QG], MDT, tag="rtmpb")
            nc.vector.tensor_copy(rtmpb, rtmpf)
            pssb = psS.tile([128, 2, QG], F32, tag="pss", name="pssb")
            nc.tensor.matmul(pssb[0:HD, 0, :], onesb, rtmpb,
                             start=True, stop=True)
            nc.vector.tensor_mul(
                U2[64 * h:64 * (h + 1), pair, QG * g:QG * (g + 1)],
                u2rn, pssb[0:HD, 0, :],
            )
            return
        rtmp = rb_pool.tile([1, QG], F32, tag="rtmp")
        nc.vector.reciprocal_approx_fast(out=rtmp, in_=rt0)
        rbt = rb_pool.tile([64, QG], F32, tag="rbt")
        nc.gpsimd.partition_broadcast(rbt, rtmp)
        nc.vector.tensor_mul(
            U2[64 * h:64 * (h + 1), pair, QG * g:QG * (g + 1)],
            u2rn, rbt,
        )

    def o_block(g, qt, dh):
        pso = acc.tile([128, QG], F32, tag="acc")
        for pair in range(NPAIR):
            nc.tensor.matmul(
                pso,
                U2[:, pair, 128 * qt:128 * (qt + 1)],
                wo_sb[:, pair, QG * dh:QG * (dh + 1)],
                start=(pair == 0), stop=(pair == NPAIR - 1),
            )
        ob = ob_pool.tile([128, QG], MDT, tag="ob")
        if nblk[0] % 2 == 0:
            nc.scalar.copy(ob, pso)
            eng = nc.scalar
        else:
            nc.vector.tensor_copy(ob, pso)
            eng = nc.sync
        nblk[0] += 1
        eng.dma_start(
            out=out[128 * qt:128 * (qt + 1), QG * dh:QG * (dh + 1)],
            in_=ob,
        )

    def o_blocks(g):
        return [(g, 4 * g + i, dh) for i in range(4) for dh in range(2)]

    # ---------------- software-pipelined main loop ----------------
    for g in range(NQG):
        vts[g] = vt_pool.tile([128, NPAIR, QG], MDT, tag="vt", name="vt")
        obl = list(o_blocks(g - 2)) if g >= 2 else []

        def ob1():
            if obl:
                o_block(*obl.pop(0))

        kts = _s_kts(g)
        for pair, nm in ((0, "q"), (1, "q"), (0, "k"), (1, "k")):
            qkv_chain(g, pair, nm)
            ob1()
        if g + 1 < NQG:
            # prefetch next group's xT now: issuing it earlier would share
            # the DMA engines with this group's xT and delay its arrival.
            nc.sync.dma_start(out=xT_sb[g + 1], in_=xTd[:, g + 1, :, :])
        if g == 0:
            s_unit(kts[0], 0)
            s_unit(kts[0], 1)
            qkv_chain(g, 0, "v")
            s_unit(kts[1], 0)
            s_unit(kts[1], 1)
            qkv_chain(g, 1, "v")
            vtrans(g, 0)
            vtrans(g, 1)
        else:
            s_unit(kts[0], 0)
            s_unit(kts[0], 1)
            ob1()
            ob1()
            s_unit(kts[1], 0)
            s_unit(kts[1], 1)
            qkv_chain(g, 0, "v")
            av_unit(g - 1, 0, 0)
            qkv_chain(g, 1, "v")
            av_unit(g - 1, 0, 1)
            s_unit(kts[2], 0)
            s_unit(kts[2], 1)
            ob1()
            av_unit(g - 1, 1, 0)
            vtrans(g, 0)
            vtrans(g, 1)
            av_unit(g - 1, 1, 1)
            s_unit(kts[3], 0)
            s_unit(kts[3], 1)
            ob1()

    # ---------------- drain ----------------
    obl = o_blocks(2)
    s_unit(14, 0)
    s_unit(14, 1)
    o_block(*obl[0])
    o_block(*obl[1])
    s_unit(15, 0)
    s_unit(15, 1)
    o_block(*obl[2])
    o_block(*obl[3])
    av_unit(3, 0, 0)
    o_block(*obl[4])
    av_unit(3, 0, 1)
    o_block(*obl[5])
    av_unit(3, 1, 0, pe_bcast=True)
    o_block(*obl[6])
    o_block(*obl[7])
    av_unit(3, 1, 1, pe_bcast=True)
    for b_ in o_blocks(3):
        o_block(*b_)
    if debug_taps is not None:
        nc.sync.dma_start(out=debug_taps[1], in_=U2)


def build(mm_dt=MM_DT):
    nc = bacc.Bacc("TRN2", target_bir_lowering=False, debug=False)
    xT = nc.dram_tensor("xT", [128, NQG, NCH, QG], mm_dt, kind="ExternalInput").ap()
    wq = nc.dram_tensor("wq", [128, NCH, 256], mm_dt, kind="ExternalInput").ap()
    wk = nc.dram_tensor("wk", [128, NCH, 256], mm_dt, kind="ExternalInput").ap()
    wv = nc.dram_tensor("wv", [128, NCH, 256], mm_dt, kind="ExternalInput").ap()
    wo = nc.dram_tensor("wo", [128, NPAIR, D], mm_dt, kind="ExternalInput").ap()
    bq = nc.dram_tensor("bq", [128, NPAIR], F32, kind="ExternalInput").ap()
    bk = nc.dram_tensor("bk", [128, NPAIR], F32, kind="ExternalInput").ap()
    bv = nc.dram_tensor("bv", [128, NPAIR], F32, kind="ExternalInput").ap()
    out = nc.dram_tensor("out", [N, D], mm_dt, kind="ExternalOutput").ap()
    with tile.TileContext(nc) as tc, ExitStack() as ctx:
        _emit(ctx, tc, (xT, wq, wk, wv, wo, bq, bk, bv, out), mm_dt)
    nc.compile()
    return nc


def shard_inputs(x, Wq, bq, Wk, bk, Wv, bv, Wo, bo):
    """Full inputs -> 8 per-core input maps, pre-arranged so every DMA line
    is contiguous per partition."""
    mdt = mybir.dt.np(MM_DT)

    def warr(W, cs):  # [1024, 256] -> [128, 8, 256]
        return np.ascontiguousarray(
            W[:, cs].reshape(NCH, 128, 256).transpose(1, 0, 2)).astype(mdt)

    xTb = [np.ascontiguousarray(
        x[b].T.reshape(NCH, 128, NQG, QG).transpose(1, 2, 0, 3)).astype(mdt)
        for b in range(2)]
    in_maps = []
    for c in range(8):
        b, hg = c // 4, c % 4
        cs = slice(256 * hg, 256 * (hg + 1))
        in_maps.append({
            "xT": xTb[b],
            "wq": warr(Wq, cs),
            "wk": warr(Wk, cs),
            "wv": warr(Wv, cs),
            "wo": np.ascontiguousarray(
                Wo[cs, :].reshape(NPAIR, 128, D).transpose(1, 0, 2)).astype(mdt),
            "bq": np.ascontiguousarray(bq[cs].reshape(NPAIR, 128).T),
            "bk": np.ascontiguousarray(bk[cs].reshape(NPAIR, 128).T),
            "bv": np.ascontiguousarray(bv[cs].reshape(NPAIR, 128).T),
        })
    return in_maps


def assemble(results, bo):
    outs = [np.asarray(r["out"], dtype=np.float32) for r in results]
    full = np.empty((2, N, D), dtype=np.float32)
    for b in range(2):
        full[b] = outs[4 * b] + outs[4 * b + 1] + outs[4 * b + 2] + outs[4 * b + 3]
        full[b] += bo[None, :]
    return full


_NC_CACHE = {}


def _get_nc():
    key = _MM_DT_NAME
    if key not in _NC_CACHE:
        _NC_CACHE[key] = build()
    return _NC_CACHE[key]


def kernel(x, Wq, bq, Wk, bk, Wv, bv, Wo, bo, _trace=False):
    x, Wq, bq, Wk, bk, Wv, bv, Wo, bo = (
        np.asarray(a, dtype=np.float32)
        for a in (x, Wq, bq, Wk, bk, Wv, bv, Wo, bo)
    )
    nc = _get_nc()
    in_maps = shard_inputs(x, Wq, bq, Wk, bk, Wv, bv, Wo, bo)
    res = run_bass_kernel_spmd(nc, in_maps, core_ids=list(range(8)), trace=_trace)
    full = assemble(res.results, bo)
    if _trace:
        kernel.last_result = res
    return full
